# revision 8
# baseline (speedup 1.0000x reference)
"""Trainium2 Bass kernel for nn_KineticModel (gnn_message_passing), v6.

Hybrid of v4 and v5b, overlapping the two input paths:

  * matvec1 consumes the DENSE 2-bit-packed relu(-S) stream (4.2 MB/core,
    v4 layout: byte bits 2q:2q+2 = value of reaction column i + 512q),
    decoded by DVE shift/and + u8->fp8 copy, streamed over the (slow)
    input-DMA path.
  * matvec2 consumes the SPARSE S stream (v5b: one GPSIMD local_scatter
    per [128,1024] bf16 tile + DVE staging copy before the PE reads it).
    Since GPSIMD does no matvec1 work in this version, it pre-scatters
    matvec2 tiles DURING matvec1 (sts pool bufs=36 of lookahead, ~9 MB
    SBUF), hiding ~1/3 of the scatter backbone under the matvec1 stream.
  * logc splits are fp8 (matvec1 operands are fp8), v splits are bf16
    (matvec2 tiles are bf16): rel err ~2.4e-3 (gate 2e-2).
"""

import sys

if "/opt/trn_rl_repo" not in sys.path:
    sys.path.insert(0, "/opt/trn_rl_repo")

import numpy as np
import ml_dtypes

import concourse.bacc as bacc
import concourse.mybir as mybir
from concourse.tile import TileContext
from concourse.bass_utils import run_bass_kernel_spmd

F32 = mybir.dt.float32
FP8 = mybir.dt.float8e4
BF16 = mybir.dt.bfloat16
U8 = mybir.dt.uint8
I16 = mybir.dt.int16
BF16_NP = ml_dtypes.bfloat16

N_SPECIES = 8192
N_RXN = 16384
N_BAL = 7680
N_CORES = 8
R_CORE = N_RXN // N_CORES        # 2048 reactions per core
SB = N_SPECIES // 128            # 64 species blocks
RB = R_CORE // 128               # 16 reaction blocks per core
PC = R_CORE // 4                 # packed bytes per s_sub tile row (512)
W = 1024                         # scatter tile width (num_elems)
NW2 = N_SPECIES // W             # 8 species windows (matvec2)
NI2 = 16                         # idx pad for s_t rows (max seen 11)

_CACHE = {}


def _build_nc():
    nc = bacc.Bacc(None, target_bir_lowering=False, debug=False)
    # 2-bit-packed relu(-S) species-major (v4 layout)
    s_sub = nc.declare_dram_parameter("s_sub", [SB, 128, PC], U8, isOutput=False)
    # sparse S reaction-major (v5b layout), r = r0 + p*16 + j
    st_idx = nc.declare_dram_parameter("st_idx", [NW2, RB, 128, NI2], I16, isOutput=False)
    st_dat = nc.declare_dram_parameter("st_dat", [NW2, RB, 128, NI2], BF16, isOutput=False)
    xa = nc.declare_dram_parameter("xa", [128, SB], F32, isOutput=False)
    xb = nc.declare_dram_parameter("xb", [128, SB], F32, isOutput=False)
    kcat = nc.declare_dram_parameter("kcat", [1, R_CORE], F32, isOutput=False)
    out = nc.declare_dram_parameter("out", [2, N_SPECIES], F32, isOutput=True)

    ts = mybir.AluOpType
    with TileContext(nc) as tc:
        with (
            tc.tile_pool(name="small", bufs=1) as small,
            tc.tile_pool(name="ssubp", bufs=6) as ssubp_pool,
            tc.tile_pool(name="ssub", bufs=4) as ssub_pool,
            tc.tile_pool(name="sti", bufs=40) as sti_pool,
            tc.tile_pool(name="sts", bufs=36) as sts_pool,
            tc.tile_pool(name="stss", bufs=4) as stss_pool,
            tc.tile_pool(name="stage", bufs=2) as stage_pool,
            tc.tile_pool(name="psv", bufs=1, space="PSUM") as psv_pool,
            tc.tile_pool(name="psd", bufs=1, space="PSUM") as psd_pool,
        ):
            # ---- logc = Ln(xa) + xb, split into interleaved hi/lo fp8 ----
            xa_t = small.tile([128, SB], F32, tag="xa")
            xb_t = small.tile([128, SB], F32, tag="xb")
            kcat_t = small.tile([1, R_CORE], F32, tag="kcat")
            nc.sync.dma_start(out=xa_t, in_=xa[:])
            nc.sync.dma_start(out=xb_t, in_=xb[:])
            nc.sync.dma_start(out=kcat_t, in_=kcat[:])

            lg = small.tile([128, SB], F32, tag="lg")
            nc.scalar.activation(lg, xa_t, mybir.ActivationFunctionType.Ln)
            logc = small.tile([128, SB], F32, tag="logc")
            nc.vector.tensor_tensor(out=logc, in0=lg, in1=xb_t, op=ts.add)

            logc_hl = small.tile([128, 2 * SB], FP8, tag="logc_hl")
            nc.vector.tensor_copy(out=logc_hl[:, 0 : 2 * SB : 2], in_=logc)
            lh_f = small.tile([128, SB], F32, tag="lh_f")
            nc.vector.tensor_copy(out=lh_f, in_=logc_hl[:, 0 : 2 * SB : 2])
            nc.vector.tensor_tensor(
                out=logc_hl[:, 1 : 2 * SB : 2], in0=logc, in1=lh_f, op=ts.subtract
            )

            # ---- pre-issue the first NPRE matvec2 sparse tile builds.
            # NPRE <= sts bufs, so none of these ever stalls waiting on
            # consumption (which needs v): GPSIMD runs them during matvec1
            # without wedging the DMA FIFOs ahead of the s_sub stream.
            NPRE = 36

            def build_st_tile(w, j):
                it = sti_pool.tile([128, NI2], I16, tag="sti")
                dt_ = sti_pool.tile([128, NI2], BF16, tag="std")
                nc.sync.dma_start(out=it, in_=st_idx[w, j])
                nc.sync.dma_start(out=dt_, in_=st_dat[w, j])
                sc = sts_pool.tile([128, W], BF16, tag="sts")
                nc.gpsimd.local_scatter(
                    sc, dt_, it, channels=128, num_elems=W, num_idxs=NI2
                )
                return sc

            sc_tiles = {}
            for k in range(NPRE):
                sc_tiles[k] = build_st_tile(k // RB, k % RB)

            # ---- matvec1: psum_v += logc_hl[sb].T @ decode(s_sub[sb]) ----
            psum_v = psv_pool.tile([2, R_CORE], F32, tag="psum_v")
            for sb in range(SB):
                pt = ssubp_pool.tile([128, PC], U8, tag="ssubp")
                nc.sync.dma_start(out=pt, in_=s_sub[sb])
                au = ssub_pool.tile([128, R_CORE], U8, tag="ssub_u")
                nc.vector.tensor_scalar(
                    out=au[:, 0:PC], in0=pt, scalar1=3, scalar2=0,
                    op0=ts.bitwise_and, op1=ts.bypass,
                )
                for qq in range(1, 4):
                    nc.vector.tensor_scalar(
                        out=au[:, qq * PC : (qq + 1) * PC], in0=pt,
                        scalar1=2 * qq, scalar2=3,
                        op0=ts.logical_shift_right, op1=ts.bitwise_and,
                    )
                at = ssub_pool.tile([128, R_CORE], FP8, tag="ssub")
                nc.vector.tensor_copy(out=at, in_=au)
                for rc in range(R_CORE // 512):
                    nc.tensor.matmul(
                        psum_v[:, rc * 512 : (rc + 1) * 512],
                        logc_hl[:, 2 * sb : 2 * sb + 2],
                        at[:, rc * 512 : (rc + 1) * 512],
                        start=(sb == 0),
                        stop=(sb == SB - 1),
                        skip_group_check=True,
                    )

            # ---- v = exp(hi_row + lo_row + kcat), hi/lo bf16 split ----
            pv = small.tile([2, R_CORE], F32, tag="pv")
            nc.vector.tensor_copy(out=pv, in_=psum_v)
            pvf = small.tile([1, 2 * R_CORE], F32, tag="pvf")
            nc.sync.dma_start(out=pvf[:, 0:R_CORE], in_=pv[0:1, :])
            nc.sync.dma_start(out=pvf[:, R_CORE : 2 * R_CORE], in_=pv[1:2, :])
            lv = small.tile([1, R_CORE], F32, tag="lv")
            nc.vector.tensor_tensor(
                out=lv, in0=pvf[:, 0:R_CORE], in1=pvf[:, R_CORE : 2 * R_CORE],
                op=ts.add,
            )
            lvk = small.tile([1, R_CORE], F32, tag="lvk")
            nc.vector.tensor_tensor(out=lvk, in0=lv, in1=kcat_t, op=ts.add)
            v_f = small.tile([1, R_CORE], F32, tag="v_f")
            nc.scalar.activation(v_f, lvk, mybir.ActivationFunctionType.Exp)

            vscr = nc.dram_tensor("vscr", [1, R_CORE], F32)
            nc.sync.dma_start(out=vscr[:], in_=v_f)
            v_pm = small.tile([128, RB], F32, tag="v_pm")
            nc.sync.dma_start(out=v_pm, in_=vscr.reshape((128, RB))[:])

            v_hl = small.tile([128, 2 * RB], BF16, tag="v_hl")
            nc.vector.tensor_copy(out=v_hl[:, 0 : 2 * RB : 2], in_=v_pm)
            vh_f = small.tile([128, RB], F32, tag="vh_f")
            nc.vector.tensor_copy(out=vh_f, in_=v_hl[:, 0 : 2 * RB : 2])
            nc.vector.tensor_tensor(
                out=v_hl[:, 1 : 2 * RB : 2], in0=v_pm, in1=vh_f, op=ts.subtract
            )

            # ---- matvec2: psum_dc += v_hl[j].T @ staged scatter tiles ----
            for w in range(NW2):
                psum_dc = psd_pool.tile([2, W], F32, tag="psum_dc")
                for j in range(RB):
                    k = w * RB + j
                    sc = sc_tiles.pop(k) if k in sc_tiles else build_st_tile(w, j)
                    scs = stss_pool.tile([128, W], BF16, tag="sts_s")
                    nc.vector.tensor_copy(out=scs, in_=sc)
                    for c in range(W // 512):
                        nc.tensor.matmul(
                            psum_dc[:, c * 512 : (c + 1) * 512],
                            v_hl[:, 2 * j : 2 * j + 2],
                            scs[:, c * 512 : (c + 1) * 512],
                            start=(j == 0),
                            stop=(j == RB - 1),
                        )
                st_out = stage_pool.tile([2, W], F32, tag="stage")
                nc.vector.tensor_copy(out=st_out, in_=psum_dc)
                nc.sync.dma_start(out=out[:, w * W : (w + 1) * W], in_=st_out)
    nc.compile()
    return nc


def _sparse_rows(mat, n_rows, width, pad):
    idx = np.full((n_rows, pad), -1, np.int16)
    val = np.zeros((n_rows, pad), np.float32)
    rr, cc = np.nonzero(mat)
    if len(rr):
        order = np.lexsort((cc, rr))
        rr, cc = rr[order], cc[order]
        starts = np.searchsorted(rr, np.arange(n_rows))
        rank = np.arange(len(rr)) - starts[rr]
        if rank.max() >= pad:
            raise ValueError(f"row nnz {rank.max() + 1} exceeds pad {pad}")
        idx[rr, rank] = cc.astype(np.int16)
        val[rr, rank] = mat[rr, cc]
    return idx, val.astype(BF16_NP)


def _prep_inputs(conc_balanced, S, balanced_species, unbalanced_species,
                 log_conc_unbalanced, log_kcat):
    """Host-side shard + layout prep (pure data movement / dtype casts)."""
    in_maps = []
    xa_full = np.ones(N_SPECIES, dtype=np.float32)
    xb_full = np.zeros(N_SPECIES, dtype=np.float32)
    xa_full[np.asarray(balanced_species)] = np.asarray(conc_balanced)
    xb_full[np.asarray(unbalanced_species)] = np.asarray(log_conc_unbalanced)
    xa_pm = np.ascontiguousarray(xa_full.reshape(SB, 128).T)
    xb_pm = np.ascontiguousarray(xb_full.reshape(SB, 128).T)

    S = np.asarray(S)
    log_kcat = np.asarray(log_kcat)
    for c in range(N_CORES):
        r0 = c * R_CORE
        sl = S[:, r0 : r0 + R_CORE].astype(np.float32)       # [8192, 2048]
        sub = np.maximum(-sl, 0.0).astype(np.uint8).reshape(SB, 128, R_CORE)
        s_sub = (
            sub[:, :, 0:PC]
            | (sub[:, :, PC : 2 * PC] << 2)
            | (sub[:, :, 2 * PC : 3 * PC] << 4)
            | (sub[:, :, 3 * PC : 4 * PC] << 6)
        )
        slT = sl.T                                           # [2048, 8192]
        st_rows = slT.reshape(128, RB, NW2, W).transpose(2, 1, 0, 3)
        i2, v2 = _sparse_rows(
            st_rows.reshape(-1, W), NW2 * RB * 128, W, NI2
        )
        kcat_pm = log_kcat[r0 : r0 + R_CORE].astype(np.float32).reshape(1, R_CORE)
        in_maps.append(
            {
                "s_sub": np.ascontiguousarray(s_sub),
                "st_idx": np.ascontiguousarray(i2.reshape(NW2, RB, 128, NI2)),
                "st_dat": np.ascontiguousarray(v2.reshape(NW2, RB, 128, NI2)),
                "xa": xa_pm,
                "xb": xb_pm,
                "kcat": np.ascontiguousarray(kcat_pm),
            }
        )
    return in_maps


def kernel(**inputs) -> np.ndarray:
    if "nc" not in _CACHE:
        _CACHE["nc"] = _build_nc()
    nc = _CACHE["nc"]
    in_maps = _prep_inputs(**inputs)
    res = run_bass_kernel_spmd(nc, in_maps, core_ids=list(range(N_CORES)))
    acc = np.zeros(N_SPECIES, dtype=np.float64)
    for c in range(N_CORES):
        o = res.results[c]["out"]
        acc += o[0].astype(np.float64) + o[1].astype(np.float64)
    return acc[:N_BAL].astype(np.float32)
